# revision 59
# baseline (speedup 1.0000x reference)
"""CAM (channel attention) module kernel for Trainium2, 8-core data-parallel.

Reference computation (per sample, C=512, HW=4096):
    v = x.reshape(C, HW)
    E = v @ v.T                                  # (C, C)
    att = softmax(rowmax(E) - E, axis=-1)        # == softmax(-E) stabilized at rowmin
    o = att @ v                                  # (C, HW)
    o = softmax(o, axis=-1)
    out = x + gamma * o
Sharding: data-parallel over batch B=16 -> 2 samples per NeuronCore.

v3: bf16 DRAM I/O.  The tolerance (2e-2) comfortably admits bf16-rounded
inputs/outputs (~2e-3), so the host casts x to bf16 and upcasts the output;
HBM traffic drops from 33.6 MB to ~25 MB per core:
- x lands in SBUF as bf16 (half the load bytes, half the SBUF);
- vb (natural-layout fp8 v for matmul 2 / the transposes) is produced by
  SWDGE cast-load DMAs straight from DRAM (bf16 -> fp8), using spare DMA
  bandwidth instead of engine time;
- all v^T transposes are fp8 identity-matmuls (FWL weight loads) feeding
  DoubleRow matmul 1; transpose pairs land in one [P,1024] PSUM tile and
  evict with a single copy;
- the final out = x + (gamma/Z2)*exp is an all-bf16 scalar_tensor_tensor on
  DVE (2x packed mode), stores are bf16;
- one shared PSUM pool ([P,1024] f32, bufs=4 = all 8 banks).
"""

import sys

if "/opt/trn_rl_repo" not in sys.path:
    sys.path.insert(0, "/opt/trn_rl_repo")

from contextlib import ExitStack

import numpy as np

P = 128
C = 512
HW = 4096
HHW = HW // 2  # 2048: half-width x tiles
S = 2  # samples per core
CB = C // P  # 4 channel blocks
NB = HW // P  # 32 spatial blocks
NT = NB // 2  # 16 DoubleRow k-pairs for matmul 1
NJ = HW // 1024  # 4 psum chunks (2 banks each) for the second matmul
N_CORES = 8

_NC = None


def _build_nc():
    import concourse.bacc as bacc
    import concourse.mybir as mybir
    import concourse.tile as tile
    from concourse.masks import make_identity

    f32 = mybir.dt.float32
    bf16 = mybir.dt.bfloat16
    fp8 = mybir.dt.float8e4
    AF = mybir.ActivationFunctionType
    ALU = mybir.AluOpType
    AX = mybir.AxisListType
    DR = mybir.MatmulPerfMode.DoubleRow

    nc = bacc.Bacc(
        "TRN2",
        target_bir_lowering=False,
        debug=False,
        num_devices=N_CORES,
        num_swdge_queues=4,
    )
    x = nc.dram_tensor("x", (S, C, HW), bf16, kind="ExternalInput").ap()
    gamma = nc.dram_tensor("gamma", (1,), f32, kind="ExternalInput").ap()
    out = nc.dram_tensor("out", (S, C, HW), bf16, kind="ExternalOutput").ap()

    with tile.TileContext(nc) as tc, ExitStack() as ctx:
        const = ctx.enter_context(tc.tile_pool(name="const", bufs=1))
        ident8 = const.tile([P, P], fp8)
        make_identity(nc, ident8)
        identb = const.tile([P, P], bf16)
        make_identity(nc, identb)
        identf = const.tile([P, P], f32)
        make_identity(nc, identf)
        gamma_sb = const.tile([P, 1], f32)
        nc.sync.dma_start(out=gamma_sb, in_=gamma.to_broadcast((P, 1)))

        xf_pool = ctx.enter_context(tc.tile_pool(name="xf_pool", bufs=8))
        vb_pool = ctx.enter_context(tc.tile_pool(name="vb_pool", bufs=4))
        vt_pool = ctx.enter_context(tc.tile_pool(name="vt_pool", bufs=NT + 2))
        att_pool = ctx.enter_context(tc.tile_pool(name="att_pool", bufs=CB + 1))
        attT_pool = ctx.enter_context(tc.tile_pool(name="attT_pool", bufs=2))
        exp_pool = ctx.enter_context(tc.tile_pool(name="exp_pool", bufs=3))
        small = ctx.enter_context(tc.tile_pool(name="small", bufs=12))
        r1_pool = ctx.enter_context(tc.tile_pool(name="r1_pool", bufs=10))
        ebuf = ctx.enter_context(tc.tile_pool(name="ebuf", bufs=4))
        # one shared PSUM pool: [P,1024] f32 bufs (2 banks each) = all 8 banks
        psum = ctx.enter_context(tc.tile_pool(name="psum", bufs=4, space="PSUM"))

        # per-sample state
        xh = [[None] * CB for _ in range(S)]
        vb2 = [[None] * (CB // 2) for _ in range(S)]
        vT2 = [[None] * NT for _ in range(S)]
        att8 = [[None] * CB for _ in range(S)]
        r1s = [[None] * CB for _ in range(S)]
        attT2 = [[None] * (CB // 2) for _ in range(S)]

        def vb_loads(s, nq, fast_first=False):
            # vb2[s][u][:, ko, :] = fp8 rows of channel chunk 2u+ko, cast
            # bf16 -> fp8 by SWDGE DMAs straight from DRAM.
            for u in range(CB // 2):
                vt_ = vb_pool.tile([P, 2, HW], fp8, tag="vb", name=f"vb2_{s}_{u}")
                vb2[s][u] = vt_
            ranges = [
                (q * (HW // nq), (q + 1) * (HW // nq)) for q in range(nq)
            ]
            if fast_first:
                # first 512 columns via sync-HWDGE bf16 staging + DVE casts:
                # skips the serial Q7 descriptor-generation latency at kernel
                # start, so the first transposes begin ~6 us earlier
                for u in range(CB // 2):
                    for ko in range(2):
                        i = 2 * u + ko
                        st = ebuf.tile([P, 512], bf16, tag="xq", name=f"xq_{s}_{i}")
                        nc.sync.dma_start(out=st, in_=x[s, i * P : (i + 1) * P, 0:512])
                        nc.vector.tensor_copy(vb2[s][u][:, ko, 0:512], st)
                ranges = [(512, HW // nq)] + ranges[1:]
            # emit in k-major chunk order so the first transposes start early
            for lo, hi in ranges:
                qs = slice(lo, hi)
                for u in range(CB // 2):
                    for ko in range(2):
                        i = 2 * u + ko
                        nc.gpsimd.dma_start(
                            out=vb2[s][u][:, ko, qs],
                            in_=x[s, i * P : (i + 1) * P, qs],
                        )

        def x_loads(s):
            # bf16 x tiles (full row-block), only consumed by the final
            # residual add.  These ride the same in-order SWDGE ring as the
            # vb cast-loads so the load stream drains in priority order
            # (vb0, vb1, x0, x1) instead of competing with them for
            # DMA-engine bandwidth.  One DMA per row-block keeps the Q7
            # descriptor-generation cost down.
            for i in range(CB):
                xt = xf_pool.tile([P, HW], bf16, tag="xf", name=f"xf_{s}_{i}")
                nc.gpsimd.dma_start(out=xt, in_=x[s, i * P : (i + 1) * P, :])
                xh[s][i] = xt

        evict_ctr = [0]

        def evict(dst, src, act_ratio=3):
            # PSUM -> SBUF eviction copy, alternating DVE / ACT.  Sample 0
            # evictions run while ACT is pinned by exp1/exp2 (1-in-3 on ACT);
            # sample 1 evictions land in ACT's post-load lull (1-in-2).
            k = evict_ctr[0]
            evict_ctr[0] += 1
            if k % act_ratio == act_ratio - 1:
                nc.scalar.copy(dst, src)
            else:
                nc.vector.tensor_copy(dst, src)

        def v_transpose_pair(s, t):
            # vT pair t (n-part, c-free) fp8: regular fp8 matmul against the
            # identity (out = v_slice.T @ I) — exact transpose into f32 PSUM,
            # FWL weight loads, no fp8 is_transpose stride-2 constraint.
            vt_ = vt_pool.tile([P, 2, C], fp8, tag="vt", name=f"vT2_{s}_{t}")
            pt = psum.tile([P, 2, C], f32, tag="ps", name=f"ptv_{s}_{t}")
            for ko in range(2):
                k = 2 * t + ko
                for i in range(CB):
                    nc.tensor.matmul(
                        pt[:, ko, i * P : (i + 1) * P],
                        lhsT=vb2[s][i // 2][:, i % 2, k * P : (k + 1) * P],
                        rhs=ident8,
                        start=True,
                        stop=True,
                    )
            evict(vt_, pt, act_ratio=3 if s == 0 else 2)
            vT2[s][t] = vt_

        def softmax1_tail(s, i, E):
            m = small.tile([P, 1], f32, tag="sm", name=f"m_{s}_{i}")
            nc.vector.tensor_reduce(m, E, axis=AX.X, op=ALU.min)
            a = att_pool.tile([P, C], bf16, tag="att", name=f"att_{s}_{i}")
            z1 = small.tile([P, 1], f32, tag="sm", name=f"z1_{s}_{i}")
            nc.scalar.activation(a, E, AF.Exp, bias=m, scale=-1.0, accum_out=z1)
            r1 = r1_pool.tile([P, 1], f32, tag="r1", name=f"r1_{s}_{i}")
            nc.vector.reciprocal(r1, z1)
            att8[s][i] = a
            r1s[s][i] = r1

        def mm1_block(s, i, Eout, t):
            nc.tensor.matmul(
                Eout,
                lhsT=vT2[s][t][:, :, i * P : (i + 1) * P],
                rhs=vT2[s][t],
                perf_mode=DR,
                start=(t == 0),
                stop=(t == NT - 1),
            )

        # --- symmetric mm1: E = v v^T, so only the upper-triangular blocks
        # are computed directly; the strictly-lower blocks of rows 1..3 are
        # exact f32 PE transposes of staged upper blocks (same accumulation
        # order both sides -> bitwise-equal mirrors).
        eb_stage = {}

        def mirrorT(dst, src):
            nc.tensor.matmul(
                dst,
                lhsT=src,
                rhs=identf,
                is_transpose=True,
                start=True,
                stop=True,
                skip_group_check=True,
            )

        def mm1_rows01(s, Ep, t):
            # row 0 full N=512; row 1 N=384 (cols 1..3)
            nc.tensor.matmul(
                Ep[:, 0, :],
                lhsT=vT2[s][t][:, :, 0:P],
                rhs=vT2[s][t],
                perf_mode=DR,
                start=(t == 0),
                stop=(t == NT - 1),
            )
            nc.tensor.matmul(
                Ep[:, 1, P:],
                lhsT=vT2[s][t][:, :, P : 2 * P],
                rhs=vT2[s][t][:, :, P:],
                perf_mode=DR,
                start=(t == 0),
                stop=(t == NT - 1),
            )

        def mm1_mirrors_A(s, Ep):
            # stage row0 cols1..3 / row1 cols2..3 to SBUF, mirror (1,0)
            eb0 = ebuf.tile([P, 384], f32, tag="eb", name=f"eb0_{s}")
            eb1 = ebuf.tile([P, 256], f32, tag="eb", name=f"eb1_{s}")
            nc.vector.tensor_copy(eb0, Ep[:, 0, P:])
            nc.scalar.copy(eb1, Ep[:, 1, 2 * P :])
            mirrorT(Ep[:, 1, 0:P], eb0[:, 0:P])
            eb_stage[s] = (eb0, eb1)

        def mm1_rows23(s, Ep, t):
            # rows 2 and 3 both direct over cols 2..3 (N=256): (3,2) is
            # computed directly so no late mirror chain gates the tails
            for j in range(2):
                nc.tensor.matmul(
                    Ep[:, j, 2 * P :],
                    lhsT=vT2[s][t][:, :, (2 + j) * P : (3 + j) * P],
                    rhs=vT2[s][t][:, :, 2 * P :],
                    perf_mode=DR,
                    start=(t == 0),
                    stop=(t == NT - 1),
                )

        def mm1_mirrors_B(s, Ep):
            eb0, eb1 = eb_stage[s]
            mirrorT(Ep[:, 0, 0:P], eb0[:, P : 2 * P])  # (2,0) <- (0,2)^T
            mirrorT(Ep[:, 0, P : 2 * P], eb1[:, 0:P])  # (2,1) <- (1,2)^T
            mirrorT(Ep[:, 1, 0:P], eb0[:, 2 * P :])  # (3,0) <- (0,3)^T
            mirrorT(Ep[:, 1, P : 2 * P], eb1[:, P:])  # (3,1) <- (1,3)^T

        def front_fused(s):
            # transposes interleaved with mm1 accumulation of row-blocks 0,1
            # (software-pipelined: mm1 for pair t-1 runs while pair t's
            # eviction is in flight); row-blocks 2,3 in a second pass.
            E01 = psum.tile([P, 2, C], f32, tag="ps", name=f"E01_{s}")
            for t in range(NT):
                v_transpose_pair(s, t)
                if t >= 1:
                    mm1_rows01(s, E01, t - 1)
            mm1_rows01(s, E01, NT - 1)
            mm1_mirrors_A(s, E01)
            for i in range(2):
                softmax1_tail(s, i, E01[:, i, :])
            E23 = psum.tile([P, 2, C], f32, tag="ps", name=f"E23_{s}")
            for t in range(NT):
                mm1_rows23(s, E23, t)
            mm1_mirrors_B(s, E23)
            for i in range(2):
                softmax1_tail(s, 2 + i, E23[:, i, :])

        def mm1_pair(s, i0):
            # standalone symmetric mm1 row-block pair (vT tiles already built)
            Ep = psum.tile([P, 2, C], f32, tag="ps", name=f"E_{s}_{i0}")
            for t in range(NT):
                if i0 == 0:
                    mm1_rows01(s, Ep, t)
                else:
                    mm1_rows23(s, Ep, t)
            if i0 == 0:
                mm1_mirrors_A(s, Ep)
            else:
                mm1_mirrors_B(s, Ep)
            for j in range(2):
                softmax1_tail(s, i0 + j, Ep[:, j, :])

        def att_transposes(s):
            # attT pairs (col-part, row-free) fp8 via bf16 PE transpose,
            # one [P,1024] eviction per pair
            for u in range(CB // 2):
                st = attT_pool.tile([P, 2, C], fp8, tag="attT", name=f"attT2_{s}_{u}")
                pt = psum.tile([P, 2, C], bf16, tag="ps", name=f"pta_{s}_{u}")
                for ko in range(2):
                    j = 2 * u + ko
                    for i in range(CB):
                        nc.tensor.transpose(
                            pt[:, ko, i * P : (i + 1) * P],
                            att8[s][i][:, j * P : (j + 1) * P],
                            identb,
                        )
                evict(st, pt)
                attT2[s][u] = st

        def finals_store(s, i, er, gz):
            xt = xh[s][i]
            # residual add in chunks: scale er by gamma/Z2 in place (packed
            # tensor_scalar), all-bf16 tensor_tensor add (packed), store the
            # chunk — stores overlap the remaining adds.  Sample 1 drains at
            # [P,1024] granularity to shorten the post-Z2 tail chain.
            nch = 4 if s == 1 else 2
            for h in range(nch):
                sl = slice(h * (HW // nch), (h + 1) * (HW // nch))
                nc.vector.tensor_scalar_mul(er[:, sl], er[:, sl], gz)
                nc.vector.tensor_tensor(
                    out=xt[:, sl], in0=er[:, sl], in1=xt[:, sl], op=ALU.add
                )
                nc.sync.dma_start(
                    out=out[s, i * P : (i + 1) * P, sl],
                    in_=xt[:, sl],
                )

        def mm2_final(s, i, defer=False):
            # o = att @ v (DoubleRow), softmax over HW (with 1/Z1 folded into
            # the exp scale), then out = x + (gamma/Z2)*exp (all-bf16 STT on
            # DVE, 2x packed mode) and bf16 store.
            er = exp_pool.tile([P, HW], bf16, tag="er", name=f"er_{s}_{i}")
            z2p = small.tile([P, NJ], f32, tag="z2p", name=f"z2p_{s}_{i}")
            for nj in range(NJ):
                o2 = psum.tile([P, 1024], f32, tag="ps", name=f"o2_{s}_{i}_{nj}")
                for hh in range(2):
                    sl = slice(nj * 1024 + hh * 512, nj * 1024 + (hh + 1) * 512)
                    for u in range(CB // 2):
                        nc.tensor.matmul(
                            o2[:, hh * 512 : (hh + 1) * 512],
                            lhsT=attT2[s][u][:, :, i * P : (i + 1) * P],
                            rhs=vb2[s][u][:, :, sl],
                            perf_mode=DR,
                            start=(u == 0),
                            stop=(u == CB // 2 - 1),
                        )
                nc.scalar.activation(
                    er[:, nj * 1024 : (nj + 1) * 1024],
                    o2,
                    AF.Exp,
                    scale=r1s[s][i],
                    accum_out=z2p[:, nj : nj + 1],
                )
            z2 = small.tile([P, 1], f32, tag="sm", name=f"z2_{s}_{i}")
            nc.vector.reduce_sum(z2, z2p, axis=AX.X)
            r2 = small.tile([P, 1], f32, tag="sm", name=f"r2_{s}_{i}")
            nc.vector.reciprocal(r2, z2)
            gz = small.tile([P, 1], f32, tag="sm", name=f"gz_{s}_{i}")
            nc.vector.tensor_scalar_mul(gz, r2, gamma_sb)
            if defer:
                # caller emits finals later, so the next mm1_pair's min
                # reductions (which gate s1's softmax -> attT -> mm2 chain)
                # aren't queued behind them on the in-order DVE
                return er, gz
            finals_store(s, i, er, gz)

        # ---- software pipeline across the two samples ----
        # one in-order load stream, highest priority first: vb0 gates the s0
        # front, vb1 gates the s1 transposes, x0/x1 are only needed by the
        # trailing residual adds
        vb_loads(0, nq=4)
        vb_loads(1, nq=2)
        x_loads(0)
        x_loads(1)
        front_fused(0)
        # s1 transposes immediately after the s0 front: their evictions land
        # on DVE/ACT before exp2(s0) piles onto those queues
        for t in range(NT):
            v_transpose_pair(1, t)
        att_transposes(0)
        # mm1(s1) pairs interleaved between mm2(s0) blocks: spreads the ACT
        # work (exp1(s1) no longer queues behind all of exp2(s0)) and lets
        # exp2(s1) start ~10 us earlier
        mm2_final(0, 0)
        d1 = mm2_final(0, 1, defer=True)
        mm1_pair(1, 0)
        finals_store(0, 1, *d1)
        d2 = mm2_final(0, 2, defer=True)
        mm1_pair(1, 2)
        att_transposes(1)
        # finals(0,2) after attT(1): its eviction on DVE must not queue
        # behind them (it gates mm2(1,0))
        finals_store(0, 2, *d2)
        # mm2(0,3) after attT(1): fills ACT's gap while attT(1)/mm2(1,0)
        # run on the PE, and exp1(s1) no longer queues behind it
        d3 = mm2_final(0, 3, defer=True)
        d4 = mm2_final(1, 0, defer=True)
        finals_store(0, 3, *d3)
        finals_store(1, 0, *d4)
        for i in range(1, CB):
            mm2_final(1, i)

    nc.compile()
    return nc


def get_nc():
    global _NC
    if _NC is None:
        _NC = _build_nc()
    return _NC


def kernel(x: np.ndarray, gamma: np.ndarray) -> np.ndarray:
    import ml_dtypes
    from concourse.bass_utils import run_bass_kernel_spmd

    B, Cx, H, W = x.shape
    assert (B, Cx, H * W) == (16, C, HW), (B, Cx, H, W)
    nc = get_nc()
    xs = np.ascontiguousarray(
        np.asarray(x, dtype=np.float32).reshape(B, Cx, H * W).astype(ml_dtypes.bfloat16)
    )
    g = np.ascontiguousarray(np.asarray(gamma, dtype=np.float32)).reshape(1)
    in_maps = [{"x": xs[S * c : S * (c + 1)], "gamma": g} for c in range(N_CORES)]
    res = run_bass_kernel_spmd(nc, in_maps, core_ids=list(range(N_CORES)))
    out = np.concatenate([res.results[c]["out"] for c in range(N_CORES)], axis=0)
    return out.astype(np.float32).reshape(B, Cx, H, W)


# revision 60
# speedup vs baseline: 1.0035x; 1.0035x over previous
"""CAM (channel attention) module kernel for Trainium2, 8-core data-parallel.

Reference computation (per sample, C=512, HW=4096):
    v = x.reshape(C, HW)
    E = v @ v.T                                  # (C, C)
    att = softmax(rowmax(E) - E, axis=-1)        # == softmax(-E) stabilized at rowmin
    o = att @ v                                  # (C, HW)
    o = softmax(o, axis=-1)
    out = x + gamma * o
Sharding: data-parallel over batch B=16 -> 2 samples per NeuronCore.

v3: bf16 DRAM I/O.  The tolerance (2e-2) comfortably admits bf16-rounded
inputs/outputs (~2e-3), so the host casts x to bf16 and upcasts the output;
HBM traffic drops from 33.6 MB to ~25 MB per core:
- x lands in SBUF as bf16 (half the load bytes, half the SBUF);
- vb (natural-layout fp8 v for matmul 2 / the transposes) is produced by
  SWDGE cast-load DMAs straight from DRAM (bf16 -> fp8), using spare DMA
  bandwidth instead of engine time;
- all v^T transposes are fp8 identity-matmuls (FWL weight loads) feeding
  DoubleRow matmul 1; transpose pairs land in one [P,1024] PSUM tile and
  evict with a single copy;
- the final out = x + (gamma/Z2)*exp is an all-bf16 scalar_tensor_tensor on
  DVE (2x packed mode), stores are bf16;
- one shared PSUM pool ([P,1024] f32, bufs=4 = all 8 banks).
"""

import sys

if "/opt/trn_rl_repo" not in sys.path:
    sys.path.insert(0, "/opt/trn_rl_repo")

from contextlib import ExitStack

import numpy as np

P = 128
C = 512
HW = 4096
HHW = HW // 2  # 2048: half-width x tiles
S = 2  # samples per core
CB = C // P  # 4 channel blocks
NB = HW // P  # 32 spatial blocks
NT = NB // 2  # 16 DoubleRow k-pairs for matmul 1
NJ = HW // 1024  # 4 psum chunks (2 banks each) for the second matmul
N_CORES = 8

_NC = None


def _build_nc():
    import concourse.bacc as bacc
    import concourse.mybir as mybir
    import concourse.tile as tile
    from concourse.masks import make_identity

    f32 = mybir.dt.float32
    bf16 = mybir.dt.bfloat16
    fp8 = mybir.dt.float8e4
    AF = mybir.ActivationFunctionType
    ALU = mybir.AluOpType
    AX = mybir.AxisListType
    DR = mybir.MatmulPerfMode.DoubleRow

    nc = bacc.Bacc(
        "TRN2",
        target_bir_lowering=False,
        debug=False,
        num_devices=N_CORES,
        num_swdge_queues=4,
    )
    x = nc.dram_tensor("x", (S, C, HW), bf16, kind="ExternalInput").ap()
    gamma = nc.dram_tensor("gamma", (1,), f32, kind="ExternalInput").ap()
    out = nc.dram_tensor("out", (S, C, HW), bf16, kind="ExternalOutput").ap()

    with tile.TileContext(nc) as tc, ExitStack() as ctx:
        const = ctx.enter_context(tc.tile_pool(name="const", bufs=1))
        ident8 = const.tile([P, P], fp8)
        make_identity(nc, ident8)
        identb = const.tile([P, P], bf16)
        make_identity(nc, identb)
        identf = const.tile([P, P], f32)
        make_identity(nc, identf)
        gamma_sb = const.tile([P, 1], f32)
        nc.sync.dma_start(out=gamma_sb, in_=gamma.to_broadcast((P, 1)))

        xf_pool = ctx.enter_context(tc.tile_pool(name="xf_pool", bufs=8))
        vb_pool = ctx.enter_context(tc.tile_pool(name="vb_pool", bufs=4))
        vt_pool = ctx.enter_context(tc.tile_pool(name="vt_pool", bufs=NT + 2))
        att_pool = ctx.enter_context(tc.tile_pool(name="att_pool", bufs=CB + 1))
        attT_pool = ctx.enter_context(tc.tile_pool(name="attT_pool", bufs=2))
        exp_pool = ctx.enter_context(tc.tile_pool(name="exp_pool", bufs=3))
        small = ctx.enter_context(tc.tile_pool(name="small", bufs=12))
        r1_pool = ctx.enter_context(tc.tile_pool(name="r1_pool", bufs=10))
        ebuf = ctx.enter_context(tc.tile_pool(name="ebuf", bufs=4))
        # one shared PSUM pool: [P,1024] f32 bufs (2 banks each) = all 8 banks
        psum = ctx.enter_context(tc.tile_pool(name="psum", bufs=4, space="PSUM"))

        # per-sample state
        xh = [[None] * CB for _ in range(S)]
        vb2 = [[None] * (CB // 2) for _ in range(S)]
        vT2 = [[None] * NT for _ in range(S)]
        att8 = [[None] * CB for _ in range(S)]
        r1s = [[None] * CB for _ in range(S)]
        attT2 = [[None] * (CB // 2) for _ in range(S)]

        def vb_loads(s, nq, fast_first=False):
            # vb2[s][u][:, ko, :] = fp8 rows of channel chunk 2u+ko, cast
            # bf16 -> fp8 by SWDGE DMAs straight from DRAM.
            for u in range(CB // 2):
                vt_ = vb_pool.tile([P, 2, HW], fp8, tag="vb", name=f"vb2_{s}_{u}")
                vb2[s][u] = vt_
            ranges = [
                (q * (HW // nq), (q + 1) * (HW // nq)) for q in range(nq)
            ]
            if fast_first:
                # first 512 columns via sync-HWDGE bf16 staging + DVE casts:
                # skips the serial Q7 descriptor-generation latency at kernel
                # start, so the first transposes begin ~6 us earlier
                for u in range(CB // 2):
                    for ko in range(2):
                        i = 2 * u + ko
                        st = ebuf.tile([P, 512], bf16, tag="xq", name=f"xq_{s}_{i}")
                        nc.sync.dma_start(out=st, in_=x[s, i * P : (i + 1) * P, 0:512])
                        nc.vector.tensor_copy(vb2[s][u][:, ko, 0:512], st)
                ranges = [(512, HW // nq)] + ranges[1:]
            # emit in k-major chunk order so the first transposes start early
            for lo, hi in ranges:
                qs = slice(lo, hi)
                for u in range(CB // 2):
                    for ko in range(2):
                        i = 2 * u + ko
                        nc.gpsimd.dma_start(
                            out=vb2[s][u][:, ko, qs],
                            in_=x[s, i * P : (i + 1) * P, qs],
                        )

        def x_loads(s):
            # bf16 x tiles (full row-block), only consumed by the final
            # residual add.  These ride the same in-order SWDGE ring as the
            # vb cast-loads so the load stream drains in priority order
            # (vb0, vb1, x0, x1) instead of competing with them for
            # DMA-engine bandwidth.  One DMA per row-block keeps the Q7
            # descriptor-generation cost down.
            for i in range(CB):
                xt = xf_pool.tile([P, HW], bf16, tag="xf", name=f"xf_{s}_{i}")
                nc.gpsimd.dma_start(out=xt, in_=x[s, i * P : (i + 1) * P, :])
                xh[s][i] = xt

        evict_ctr = [0]

        def evict(dst, src, act_ratio=3):
            # PSUM -> SBUF eviction copy, alternating DVE / ACT.  Sample 0
            # evictions run while ACT is pinned by exp1/exp2 (1-in-3 on ACT);
            # sample 1 evictions land in ACT's post-load lull (1-in-2).
            k = evict_ctr[0]
            evict_ctr[0] += 1
            if k % act_ratio == act_ratio - 1:
                nc.scalar.copy(dst, src)
            else:
                nc.vector.tensor_copy(dst, src)

        def v_transpose_pair(s, t):
            # vT pair t (n-part, c-free) fp8: regular fp8 matmul against the
            # identity (out = v_slice.T @ I) — exact transpose into f32 PSUM,
            # FWL weight loads, no fp8 is_transpose stride-2 constraint.
            vt_ = vt_pool.tile([P, 2, C], fp8, tag="vt", name=f"vT2_{s}_{t}")
            pt = psum.tile([P, 2, C], f32, tag="ps", name=f"ptv_{s}_{t}")
            for ko in range(2):
                k = 2 * t + ko
                for i in range(CB):
                    nc.tensor.matmul(
                        pt[:, ko, i * P : (i + 1) * P],
                        lhsT=vb2[s][i // 2][:, i % 2, k * P : (k + 1) * P],
                        rhs=ident8,
                        start=True,
                        stop=True,
                    )
            evict(vt_, pt, act_ratio=3 if s == 0 else 2)
            vT2[s][t] = vt_

        def softmax1_tail(s, i, E):
            m = small.tile([P, 1], f32, tag="sm", name=f"m_{s}_{i}")
            nc.vector.tensor_reduce(m, E, axis=AX.X, op=ALU.min)
            a = att_pool.tile([P, C], bf16, tag="att", name=f"att_{s}_{i}")
            z1 = small.tile([P, 1], f32, tag="sm", name=f"z1_{s}_{i}")
            nc.scalar.activation(a, E, AF.Exp, bias=m, scale=-1.0, accum_out=z1)
            r1 = r1_pool.tile([P, 1], f32, tag="r1", name=f"r1_{s}_{i}")
            nc.vector.reciprocal(r1, z1)
            att8[s][i] = a
            r1s[s][i] = r1

        def mm1_block(s, i, Eout, t):
            nc.tensor.matmul(
                Eout,
                lhsT=vT2[s][t][:, :, i * P : (i + 1) * P],
                rhs=vT2[s][t],
                perf_mode=DR,
                start=(t == 0),
                stop=(t == NT - 1),
            )

        # --- symmetric mm1: E = v v^T, so only the upper-triangular blocks
        # are computed directly; the strictly-lower blocks of rows 1..3 are
        # exact f32 PE transposes of staged upper blocks (same accumulation
        # order both sides -> bitwise-equal mirrors).
        eb_stage = {}

        def mirrorT(dst, src):
            nc.tensor.matmul(
                dst,
                lhsT=src,
                rhs=identf,
                is_transpose=True,
                start=True,
                stop=True,
                skip_group_check=True,
            )

        def mm1_rows01(s, Ep, t):
            # row 0 full N=512; row 1 N=384 (cols 1..3)
            nc.tensor.matmul(
                Ep[:, 0, :],
                lhsT=vT2[s][t][:, :, 0:P],
                rhs=vT2[s][t],
                perf_mode=DR,
                start=(t == 0),
                stop=(t == NT - 1),
            )
            nc.tensor.matmul(
                Ep[:, 1, P:],
                lhsT=vT2[s][t][:, :, P : 2 * P],
                rhs=vT2[s][t][:, :, P:],
                perf_mode=DR,
                start=(t == 0),
                stop=(t == NT - 1),
            )

        def mm1_mirrors_A(s, Ep):
            # stage row0 cols1..3 / row1 cols2..3 to SBUF, mirror (1,0)
            eb0 = ebuf.tile([P, 384], f32, tag="eb", name=f"eb0_{s}")
            eb1 = ebuf.tile([P, 256], f32, tag="eb", name=f"eb1_{s}")
            nc.vector.tensor_copy(eb0, Ep[:, 0, P:])
            nc.scalar.copy(eb1, Ep[:, 1, 2 * P :])
            mirrorT(Ep[:, 1, 0:P], eb0[:, 0:P])
            eb_stage[s] = (eb0, eb1)

        def mm1_rows23(s, Ep, t):
            # rows 2 and 3 both direct over cols 2..3 (N=256): (3,2) is
            # computed directly so no late mirror chain gates the tails
            for j in range(2):
                nc.tensor.matmul(
                    Ep[:, j, 2 * P :],
                    lhsT=vT2[s][t][:, :, (2 + j) * P : (3 + j) * P],
                    rhs=vT2[s][t][:, :, 2 * P :],
                    perf_mode=DR,
                    start=(t == 0),
                    stop=(t == NT - 1),
                )

        def mm1_mirrors_B(s, Ep):
            eb0, eb1 = eb_stage[s]
            mirrorT(Ep[:, 0, 0:P], eb0[:, P : 2 * P])  # (2,0) <- (0,2)^T
            mirrorT(Ep[:, 0, P : 2 * P], eb1[:, 0:P])  # (2,1) <- (1,2)^T
            mirrorT(Ep[:, 1, 0:P], eb0[:, 2 * P :])  # (3,0) <- (0,3)^T
            mirrorT(Ep[:, 1, P : 2 * P], eb1[:, P:])  # (3,1) <- (1,3)^T

        def front_fused(s):
            # transposes interleaved with mm1 accumulation of row-blocks 0,1
            # (software-pipelined: mm1 for pair t-1 runs while pair t's
            # eviction is in flight); row-blocks 2,3 in a second pass.
            E01 = psum.tile([P, 2, C], f32, tag="ps", name=f"E01_{s}")
            for t in range(NT):
                v_transpose_pair(s, t)
                if t >= 1:
                    mm1_rows01(s, E01, t - 1)
            mm1_rows01(s, E01, NT - 1)
            mm1_mirrors_A(s, E01)
            for i in range(2):
                softmax1_tail(s, i, E01[:, i, :])
            E23 = psum.tile([P, 2, C], f32, tag="ps", name=f"E23_{s}")
            for t in range(NT):
                mm1_rows23(s, E23, t)
            mm1_mirrors_B(s, E23)
            for i in range(2):
                softmax1_tail(s, 2 + i, E23[:, i, :])

        def mm1_pair(s, i0):
            # standalone symmetric mm1 row-block pair (vT tiles already built)
            Ep = psum.tile([P, 2, C], f32, tag="ps", name=f"E_{s}_{i0}")
            for t in range(NT):
                if i0 == 0:
                    mm1_rows01(s, Ep, t)
                else:
                    mm1_rows23(s, Ep, t)
            if i0 == 0:
                mm1_mirrors_A(s, Ep)
            else:
                mm1_mirrors_B(s, Ep)
            for j in range(2):
                softmax1_tail(s, i0 + j, Ep[:, j, :])

        def att_transposes(s):
            # attT pairs (col-part, row-free) fp8 via bf16 PE transpose,
            # one [P,1024] eviction per pair
            for u in range(CB // 2):
                st = attT_pool.tile([P, 2, C], fp8, tag="attT", name=f"attT2_{s}_{u}")
                pt = psum.tile([P, 2, C], bf16, tag="ps", name=f"pta_{s}_{u}")
                for ko in range(2):
                    j = 2 * u + ko
                    for i in range(CB):
                        nc.tensor.transpose(
                            pt[:, ko, i * P : (i + 1) * P],
                            att8[s][i][:, j * P : (j + 1) * P],
                            identb,
                        )
                evict(st, pt)
                attT2[s][u] = st

        def finals_store(s, i, er, gz):
            xt = xh[s][i]
            # residual add in chunks: scale er by gamma/Z2 in place (packed
            # tensor_scalar), all-bf16 tensor_tensor add (packed), store the
            # chunk — stores overlap the remaining adds.  Sample 1 drains at
            # [P,1024] granularity to shorten the post-Z2 tail chain.
            nch = 4 if s == 1 else 2
            for h in range(nch):
                sl = slice(h * (HW // nch), (h + 1) * (HW // nch))
                nc.vector.tensor_scalar_mul(er[:, sl], er[:, sl], gz)
                nc.vector.tensor_tensor(
                    out=xt[:, sl], in0=er[:, sl], in1=xt[:, sl], op=ALU.add
                )
                nc.sync.dma_start(
                    out=out[s, i * P : (i + 1) * P, sl],
                    in_=xt[:, sl],
                )

        def mm2_final(s, i, defer=False):
            # o = att @ v (DoubleRow), softmax over HW (with 1/Z1 folded into
            # the exp scale), then out = x + (gamma/Z2)*exp (all-bf16 STT on
            # DVE, 2x packed mode) and bf16 store.
            er = exp_pool.tile([P, HW], bf16, tag="er", name=f"er_{s}_{i}")
            z2p = small.tile([P, NJ], f32, tag="z2p", name=f"z2p_{s}_{i}")
            for nj in range(NJ):
                o2 = psum.tile([P, 1024], f32, tag="ps", name=f"o2_{s}_{i}_{nj}")
                for hh in range(2):
                    sl = slice(nj * 1024 + hh * 512, nj * 1024 + (hh + 1) * 512)
                    for u in range(CB // 2):
                        nc.tensor.matmul(
                            o2[:, hh * 512 : (hh + 1) * 512],
                            lhsT=attT2[s][u][:, :, i * P : (i + 1) * P],
                            rhs=vb2[s][u][:, :, sl],
                            perf_mode=DR,
                            start=(u == 0),
                            stop=(u == CB // 2 - 1),
                        )
                nc.scalar.activation(
                    er[:, nj * 1024 : (nj + 1) * 1024],
                    o2,
                    AF.Exp,
                    scale=r1s[s][i],
                    accum_out=z2p[:, nj : nj + 1],
                )
            z2 = small.tile([P, 1], f32, tag="sm", name=f"z2_{s}_{i}")
            nc.vector.reduce_sum(z2, z2p, axis=AX.X)
            r2 = small.tile([P, 1], f32, tag="sm", name=f"r2_{s}_{i}")
            nc.vector.reciprocal(r2, z2)
            gz = small.tile([P, 1], f32, tag="sm", name=f"gz_{s}_{i}")
            nc.vector.tensor_scalar_mul(gz, r2, gamma_sb)
            if defer:
                # caller emits finals later, so the next mm1_pair's min
                # reductions (which gate s1's softmax -> attT -> mm2 chain)
                # aren't queued behind them on the in-order DVE
                return er, gz
            finals_store(s, i, er, gz)

        # ---- software pipeline across the two samples ----
        # one in-order load stream, highest priority first: vb0 gates the s0
        # front, vb1 gates the s1 transposes, x0/x1 are only needed by the
        # trailing residual adds
        vb_loads(0, nq=4)
        vb_loads(1, nq=2)
        x_loads(0)
        x_loads(1)
        front_fused(0)
        # s1 transposes immediately after the s0 front: their evictions land
        # on DVE/ACT before exp2(s0) piles onto those queues
        for t in range(NT):
            v_transpose_pair(1, t)
        att_transposes(0)
        # mm1(s1) pairs interleaved between mm2(s0) blocks: spreads the ACT
        # work (exp1(s1) no longer queues behind all of exp2(s0)) and lets
        # exp2(s1) start ~10 us earlier
        mm2_final(0, 0)
        d1 = mm2_final(0, 1, defer=True)
        mm1_pair(1, 0)
        finals_store(0, 1, *d1)
        d2 = mm2_final(0, 2, defer=True)
        mm1_pair(1, 2)
        finals_store(0, 2, *d2)
        att_transposes(1)
        # mm2(0,3) after attT(1): fills ACT's gap while attT(1)/mm2(1,0)
        # run on the PE, and exp1(s1) no longer queues behind it
        mm2_final(0, 3)
        for i in range(CB):
            mm2_final(1, i)

    nc.compile()
    return nc


def get_nc():
    global _NC
    if _NC is None:
        _NC = _build_nc()
    return _NC


def kernel(x: np.ndarray, gamma: np.ndarray) -> np.ndarray:
    import ml_dtypes
    from concourse.bass_utils import run_bass_kernel_spmd

    B, Cx, H, W = x.shape
    assert (B, Cx, H * W) == (16, C, HW), (B, Cx, H, W)
    nc = get_nc()
    xs = np.ascontiguousarray(
        np.asarray(x, dtype=np.float32).reshape(B, Cx, H * W).astype(ml_dtypes.bfloat16)
    )
    g = np.ascontiguousarray(np.asarray(gamma, dtype=np.float32)).reshape(1)
    in_maps = [{"x": xs[S * c : S * (c + 1)], "gamma": g} for c in range(N_CORES)]
    res = run_bass_kernel_spmd(nc, in_maps, core_ids=list(range(N_CORES)))
    out = np.concatenate([res.results[c]["out"] for c in range(N_CORES)], axis=0)
    return out.astype(np.float32).reshape(B, Cx, H, W)
